# revision 57
# baseline (speedup 1.0000x reference)
"""GCNCombiner Trainium2 kernel — 8-core batch-parallel Bass/Tile implementation.

Math (reference):
  hs0 = x_flat @ w_pool0.T + b_pool0          (B, PS, NJ)
  q1  = mean_o(w_q @ hs0 + b_q) = u_q . hs0 + mean(b_q)   (B, NJ)
  k1  likewise
  A1  = adj1 + tanh(q1[:,None] - k1[None,:]) * alpha      (B, NJ, NJ)
  hs1 = w_c1 @ hs0 + b_c1                     (B, PS, NJ)
  hs2 = hs1 @ A1                              (B, PS, NJ)
  BN over (b, j) per channel; pool with w_pool1; classifier.

Because BN is a per-channel affine map s*h+t, the final output only needs
  r[b,c]    = sum_j hs2[b,c,j] * w_pool1[j]
  ssum[c]   = sum_{b,j} hs2[b,c,j]
  ssq[c]    = sum_{b,j} hs2[b,c,j]^2
Each core computes these for its 4 batches; the 8-way reduction of
ssum/ssq (the BN batch-stats all-reduce) and the tiny (32x1536)@(1536x200)
classifier run on the host during the gather/unshard step.

Device schedule (v2): x is host-swizzled n-block-major so pool0 runs
n-outer/k-inner — each 512-wide output bank completes after 16 matmuls,
and its transposes + conv1 chunks pipeline INSIDE the rest of pool0
instead of serializing after it (batch 0 keeps conv1 at the tail since
wc1T is still in flight).  q/k ride conv1 as a 4th 2-column moving
chunk against the same stationary hs0 chunk.  The previous batch's
hs2/stats matmuls weave into this batch's stream as PE filler.  All
PSUM->SBUF copies alternate scalar/vector/gpsimd so no single engine's
queue gates the PE.  DMA order: pT, x0 (6 chunks), wc1T (12 chunks),
x1, x2, x3 — consumption order matches arrival order everywhere.
"""

import numpy as np

import concourse.bacc as bacc
import concourse.mybir as mybir
import concourse.tile as tile
from concourse.bass_utils import run_bass_kernel_spmd

# problem shapes (hardcoded per contract)
B, PS, H, W = 32, 1536, 32, 64
S = H * W                # 2048 selects
NJ = 128                 # joints
QK = PS // 4
NC = 200
BN_EPS = 1e-5

NCORES = 8
PB = B // NCORES         # batches per core = 4
SK = S // 128            # 16 s-chunks
CK = PS // 128           # 12 c-chunks
NK = PS // 512           # 3 free-dim chunks of 512
NBLK = SK * 512          # 8192 cols per n-block of x

F16 = mybir.dt.float16
F32 = mybir.dt.float32
AF = mybir.ActivationFunctionType

TRACE = False            # set True (e.g. from test.py) to profile via NTFF
LAST_EXEC_NS = None
TMPDIR = None
_CACHE = {}


def _build_nc(with_bc1=True):
    nc = bacc.Bacc("TRN2", target_bir_lowering=False, debug=False,
                   num_devices=NCORES)

    d = {}
    # layouts pre-swizzled on host so each SBUF partition's bytes are one
    # contiguous DRAM run (large DMA descriptors -> near-peak HBM bandwidth).
    # x is n-block-major: [b, p, n, k, 512] so pool0 can run n-outer.
    d["xh"] = nc.dram_tensor("xh", [PB, 128, NK * NBLK], F16,
                             kind="ExternalInput").ap()
    d["pT"] = nc.dram_tensor("pT", [128, SK * NJ], F16, kind="ExternalInput").ap()
    d["wc1T"] = nc.dram_tensor("wc1T", [128, CK * PS], F16,
                               kind="ExternalInput").ap()
    d["ukq"] = nc.dram_tensor("ukq", [128, CK * 2], F16, kind="ExternalInput").ap()
    d["onesw1"] = nc.dram_tensor("onesw1", [128, 2], F16, kind="ExternalInput").ap()
    d["adj"] = nc.dram_tensor("adj", [NJ, NJ], F32, kind="ExternalInput").ap()
    d["ident"] = nc.dram_tensor("ident", [128, 128], F16, kind="ExternalInput").ap()
    d["ident32"] = nc.dram_tensor("ident32", [128, 128], F32, kind="ExternalInput").ap()
    d["ones1_16"] = nc.dram_tensor("ones1_16", [1, 128], F16, kind="ExternalInput").ap()
    d["ones1_32"] = nc.dram_tensor("ones1_32", [1, 128], F32, kind="ExternalInput").ap()
    d["bc1"] = nc.dram_tensor("bc1", [1, PS], F16, kind="ExternalInput").ap()
    d["bp0"] = nc.dram_tensor("bp0", [128, 1], F32, kind="ExternalInput").ap()
    d["w1col"] = nc.dram_tensor("w1col", [128, 1], F32, kind="ExternalInput").ap()
    d["c0"] = nc.dram_tensor("c0", [128, 1], F32, kind="ExternalInput").ap()
    d["alphac"] = nc.dram_tensor("alphac", [128, 1], F32, kind="ExternalInput").ap()

    # per batch: [r, ssum, ssq] concatenated along the free dim
    rss_out = nc.dram_tensor("rss_out", [PB, 3, PS], F32,
                             kind="ExternalOutput").ap()

    with tile.TileContext(nc) as tc:
        with tc.tile_pool(name="const", bufs=1) as cp, \
             tc.tile_pool(name="xp", bufs=2) as xp, \
             tc.tile_pool(name="work", bufs=2) as wp, \
             tc.tile_pool(name="sm", bufs=2) as smp, \
             tc.tile_pool(name="rp", bufs=2) as rp, \
             tc.tile_pool(name="h2p", bufs=3) as wph, \
             tc.tile_pool(name="mm", bufs=3, space="PSUM") as pmm, \
             tc.tile_pool(name="cv", bufs=3, space="PSUM") as pcv, \
             tc.tile_pool(name="tr", bufs=1, space="PSUM") as ptr, \
             tc.tile_pool(name="aux", bufs=1, space="PSUM") as paux:

            # ---- DMA order matters: pT first, then batch-0 x, then wc1T
            # (12 chunk-DMAs for fine-grained conv1 pacing), then x1..x3 ----
            pT_sb = cp.tile([128, SK * NJ], F16, tag="pT")
            nc.sync.dma_start(out=pT_sb[:], in_=d["pT"])

            # batch-0 x and wc1 alternate between the sync and scalar HWDGE
            # rings: two descriptor-generator streams fill the 16 DMA engines
            # faster during the cold-start ramp.
            x_tiles = []
            x0 = xp.tile([128, NK * NBLK], F16, tag="x", name="x_sb_pre0")
            hf = NBLK // 4
            for ci in range(12):
                nc.sync.dma_start(out=x0[:, ci * hf:(ci + 1) * hf],
                                  in_=d["xh"][0, :, ci * hf:(ci + 1) * hf])
            x_tiles.append(x0)

            wc1_sb = cp.tile([128, CK * PS], F16, tag="wc1")
            for k in range(CK):
                nc.sync.dma_start(out=wc1_sb[:, k * PS:(k + 1) * PS],
                                  in_=d["wc1T"][:, k * PS:(k + 1) * PS])

            for b in range(1, PB):
                xb = xp.tile([128, NK * NBLK], F16, tag="x", name=f"x_sb{b}")
                for ci in range(12):
                    nc.sync.dma_start(out=xb[:, ci * hf:(ci + 1) * hf],
                                      in_=d["xh"][b, :, ci * hf:(ci + 1) * hf])
                x_tiles.append(xb)

            # small constants ride the gpsimd (SWDGE) queue in parallel
            ukq_sb = cp.tile([128, CK * 2], F16, tag="ukq")
            nc.gpsimd.dma_start(out=ukq_sb[:], in_=d["ukq"])
            onesw1_sb = cp.tile([128, 2], F16, tag="onesw1")
            nc.gpsimd.dma_start(out=onesw1_sb[:], in_=d["onesw1"])
            adj_sb = cp.tile([NJ, NJ], F32, tag="adj")
            nc.gpsimd.dma_start(out=adj_sb[:], in_=d["adj"])
            ident_sb = cp.tile([128, 128], F16, tag="ident")
            nc.gpsimd.dma_start(out=ident_sb[:], in_=d["ident"])
            ident32_sb = cp.tile([128, 128], F32, tag="ident32")
            nc.gpsimd.dma_start(out=ident32_sb[:], in_=d["ident32"])
            ones16_sb = cp.tile([1, 128], F16, tag="ones16")
            nc.gpsimd.dma_start(out=ones16_sb[:], in_=d["ones1_16"])
            ones32_sb = cp.tile([1, 128], F32, tag="ones32")
            nc.gpsimd.dma_start(out=ones32_sb[:], in_=d["ones1_32"])
            bc1_sb = cp.tile([1, PS], F16, tag="bc1")
            nc.gpsimd.dma_start(out=bc1_sb[:], in_=d["bc1"])
            bp0_sb = cp.tile([128, 1], F32, tag="bp0")
            nc.gpsimd.dma_start(out=bp0_sb[:], in_=d["bp0"])
            w1col_sb = cp.tile([128, 1], F32, tag="w1col")
            nc.gpsimd.dma_start(out=w1col_sb[:], in_=d["w1col"])
            c0_sb = cp.tile([128, 1], F32, tag="c0")
            nc.gpsimd.dma_start(out=c0_sb[:], in_=d["c0"])
            alpha_sb = cp.tile([128, 1], F32, tag="alphac")
            nc.gpsimd.dma_start(out=alpha_sb[:], in_=d["alphac"])

            # HAM warmup: the PE would otherwise idle ~9us waiting for the
            # first DMAs; dummy matmuls on a memset tile bring the clock
            # gate up before the real stream starts.
            wu_sb = cp.tile([128, 512], F16, tag="wu")
            nc.vector.memset(wu_sb[:], 0.0)
            for wi in range(24):
                pw = pmm.tile([128, 512], F32, tag="mmt", name=f"wu{wi}")
                nc.tensor.matmul(pw[:], wu_sb[:, 0:128], wu_sb[:],
                                 start=True, stop=True)

            # single-bank PSUM scratch: transposes rotate through 8 slices
            # of one persistent [128,1024]f16 tile; the small matmul outputs
            # (pkq accum, k-row broadcast, kq transpose) carve disjoint
            # column ranges of one persistent [128,512]f32 tile.  Sub-tile
            # range tracking gives each slice independent dependencies.
            trbig = ptr.tile([128, 1024], F16, tag="trt", name="trbig")
            auxt = paux.tile([128, 512], F32, tag="auxt", name="auxt")
            tr_slot = [0]

            state = [None] * PB     # (hs1T_sb, a1_sb) per batch
            h2state = [None] * PB   # (h2cs, sqcs) per batch

            def emit_h2(b, nlist=range(NK)):
                """hs2T chunk matmuls + PSUM->SBUF copies for batch b.  Each
                chunk's h2/square copies ride different engines so a chunk is
                consumable ~0.7us after its matmul."""
                hs1T_sb, a1_sb = state[b]
                if h2state[b] is None:
                    h2state[b] = ([None] * NK, [None] * NK)
                h2cs, sqcs = h2state[b]
                for n in nlist:
                    ph = pmm.tile([128, 512], F32, tag="mmt", name=f"h2_{b}_{n}")
                    nc.tensor.matmul(ph[:], a1_sb[:],
                                     hs1T_sb[:, n * 512:(n + 1) * 512],
                                     start=True, stop=True)
                    h2_sb = wph.tile([128, 512], F16, tag="h2c",
                                     name=f"h2c{b}_{n}")
                    sq_sb = wph.tile([128, 512], F16, tag="sqc",
                                     name=f"sqc{b}_{n}")
                    if n % 2 == 0:
                        nc.vector.tensor_copy(h2_sb[:], ph[:])
                        nc.scalar.activation(sq_sb[:], ph[:], AF.Square)
                    else:
                        nc.scalar.activation(h2_sb[:], ph[:], AF.Copy)
                        nc.vector.tensor_tensor(sq_sb[:], h2_sb[:], h2_sb[:],
                                                mybir.AluOpType.mult)
                    h2cs[n] = h2_sb
                    sqcs[n] = sq_sb

            _stats_sb = {}

            def emit_stats_n(b, n):
                h2cs, sqcs = h2state[b]
                if b not in _stats_sb:
                    _stats_sb[b] = (
                        rp.tile([2, PS], F32, tag="sr", name=f"sr{b}"),
                        rp.tile([1, PS], F32, tag="ssq", name=f"ssq{b}"))
                sr_sb, ssq_sb = _stats_sb[b]
                sl = slice(n * 512, (n + 1) * 512)
                prs = pmm.tile([2, 512], F32, tag="mmt", name=f"prs{b}_{n}")
                nc.tensor.matmul(prs[:], onesw1_sb[:], h2cs[n][:],
                                 start=True, stop=True)
                pq2 = pmm.tile([1, 512], F32, tag="mmt", name=f"pq2{b}_{n}")
                nc.tensor.matmul(pq2[:], onesw1_sb[:, 0:1], sqcs[n][:],
                                 start=True, stop=True)
                nc.vector.tensor_copy(sr_sb[:, sl], prs[:])
                nc.scalar.activation(ssq_sb[:, sl], pq2[:], AF.Copy)

            def stats_dma_slice(b, n):
                """Per-slice output DMA for the last batch: descriptors are
                generated while the PE still runs, and the final pair rides
                two queues in parallel."""
                sr_sb, ssq_sb = _stats_sb[b]
                sl = slice(n * 512, (n + 1) * 512)
                nc.sync.dma_start(out=rss_out[b, 0:2, sl], in_=sr_sb[:, sl])
                nc.gpsimd.dma_start(out=rss_out[b, 2:3, sl], in_=ssq_sb[:, sl])

            def stats_dma(b, last=False):
                sr_sb, ssq_sb = _stats_sb[b]
                # rss_out rows: 0 = ssum, 1 = r, 2 = ssq (sr rows are [ssum, r])
                eng = nc.sync if last else nc.gpsimd
                eng.dma_start(out=rss_out[b, 0:2, :], in_=sr_sb[:])
                eng.dma_start(out=rss_out[b, 2:3, :], in_=ssq_sb[:])

            def emit_stats(b):
                """r/ssum/ssq reductions + output DMA for batch b."""
                for n in range(NK):
                    emit_stats_n(b, n)
                stats_dma(b)

            def emit_h2_aux(b, n):
                """Single hs2T chunk routed through the shared aux PSUM bank:
                lets the previous batch's tail run inside this batch's
                DMA-paced pool0 windows without extra PSUM banks."""
                hs1T_sb, a1_sb = state[b]
                if h2state[b] is None:
                    h2state[b] = ([None] * NK, [None] * NK)
                h2cs, sqcs = h2state[b]
                ph = auxt[:]
                nc.tensor.matmul(ph, a1_sb[:],
                                 hs1T_sb[:, n * 512:(n + 1) * 512],
                                 start=True, stop=True)
                h2_sb = wph.tile([128, 512], F16, tag="h2c", name=f"h2c{b}_{n}")
                sq_sb = wph.tile([128, 512], F16, tag="sqc", name=f"sqc{b}_{n}")
                if n % 2 == 0:
                    nc.vector.tensor_copy(h2_sb[:], ph)
                    nc.scalar.activation(sq_sb[:], ph, AF.Square)
                else:
                    nc.scalar.activation(h2_sb[:], ph, AF.Copy)
                    nc.vector.tensor_tensor(sq_sb[:], h2_sb[:], h2_sb[:],
                                            mybir.AluOpType.mult)
                h2cs[n] = h2_sb
                sqcs[n] = sq_sb

            def emit_stats_aux(b, n):
                h2cs, sqcs = h2state[b]
                if b not in _stats_sb:
                    _stats_sb[b] = (
                        rp.tile([2, PS], F32, tag="sr", name=f"sr{b}"),
                        rp.tile([1, PS], F32, tag="ssq", name=f"ssq{b}"))
                sr_sb, ssq_sb = _stats_sb[b]
                sl = slice(n * 512, (n + 1) * 512)
                prs = auxt[0:2, :]
                nc.tensor.matmul(prs, onesw1_sb[:], h2cs[n][:],
                                 start=True, stop=True)
                pq2 = auxt[32:33, :]
                nc.tensor.matmul(pq2, onesw1_sb[:, 0:1], sqcs[n][:],
                                 start=True, stop=True)
                nc.vector.tensor_copy(sr_sb[:, sl], prs)
                nc.scalar.activation(ssq_sb[:, sl], pq2, AF.Copy)



            def run_iter(b):
                """One batch: n-outer pool0 with transposes (and for b>0 the
                conv1 chunks + previous batch's tail) woven into the stream."""
                x_sb = x_tiles[b]

                hs0T_cs = [None] * NK     # SBUF hs0T 512-col groups
                hs0_sb = wp.tile([128, CK * NJ], F16, tag="hs0", name=f"hs0_{b}")
                hs1T_sb = wp.tile([128, PS], F16, tag="hs1T", name=f"hs1T{b}")
                pcs = [pcv.tile([128, 512], F32, tag="cvt", name=f"c1_{b}_{n}")
                       for n in range(NK)]
                pkq = auxt[:, 0:2]
                pss = [pmm.tile([128, 512], F32, tag="mmt", name=f"p0_{b}_{n}")
                       for n in range(NK)]

                def pool0_group(n, klo, khi):
                    for k in range(klo, khi):
                        nc.tensor.matmul(
                            pss[n][:],
                            pT_sb[:, k * NJ:(k + 1) * NJ],
                            x_sb[:, n * NBLK + k * 512: n * NBLK + k * 512 + 512],
                            start=(k == 0), stop=(k == SK - 1))

                def copy_hs0T(n):
                    t = wp.tile([128, 512], F16, tag=f"hs0T{n}",
                                name=f"hs0T{b}_{n}")
                    # fold b_pool0 (per-partition j) into the PSUM->SBUF copy
                    nc.vector.tensor_scalar_add(t[:], pss[n][:], bp0_sb[:])
                    hs0T_cs[n] = t

                def tr_chunk(k):
                    s = tr_slot[0] % 8
                    tr_slot[0] += 1
                    pt = trbig[:, s * 128:(s + 1) * 128]
                    nc.tensor.transpose(
                        pt,
                        hs0T_cs[k // 4][:, (k % 4) * 128:(k % 4) * 128 + 128],
                        ident_sb[:])
                    if k % 2 == 0:
                        nc.vector.tensor_copy(hs0_sb[:, k * NJ:(k + 1) * NJ], pt)
                    else:
                        nc.scalar.activation(hs0_sb[:, k * NJ:(k + 1) * NJ],
                                             pt, AF.Copy)

                def conv1_chunk(k):
                    st = hs0_sb[:, k * NJ:(k + 1) * NJ]
                    for n in range(NK):
                        nc.tensor.matmul(
                            pcs[n][:], st,
                            wc1_sb[:, k * PS + n * 512: k * PS + n * 512 + 512],
                            start=(k == 0),
                            stop=(not with_bc1 and k == CK - 1))

                def kq_burst():
                    # q/k projections: 12 cheap 2-col matmuls on the same
                    # stationaries conv1 uses; run as one burst the moment
                    # the last transpose lands so the A1 chain can overlap
                    # the remaining conv1 chunks.
                    for k in range(CK):
                        nc.tensor.matmul(pkq, hs0_sb[:, k * NJ:(k + 1) * NJ],
                                         ukq_sb[:, 2 * k:2 * k + 2],
                                         start=(k == 0), stop=(k == CK - 1))

                # ---- A1 chain pieces (PE ops woven into conv1 below) ----
                kq2_sb = smp.tile([128, 2], F32, tag="kq2", name=f"kq2{b}")
                qcol_sb = smp.tile([128, 1], F32, tag="qcol", name=f"qcol{b}")
                krow_sb = smp.tile([1, 128], F32, tag="krow", name=f"krow{b}")
                tanh_sb = smp.tile([128, 128], F32, tag="tanh", name=f"tanh{b}")
                a1_sb = smp.tile([NJ, NJ], F16, tag="a1", name=f"a1_{b}")
                pqt = auxt[0:2, 130:258]
                pbc = auxt[:, 2:130]

                def a1_stage0():
                    # pkq cols: [k0, q0]; qcol = q0 + (mean(b_q)-mean(b_k))
                    nc.vector.tensor_copy(kq2_sb[:], pkq)
                    nc.scalar.activation(qcol_sb[:], pkq[:, 1:2], AF.Identity,
                                         bias=c0_sb[:])

                def a1_stage1():
                    nc.tensor.transpose(pqt, kq2_sb[:], ident32_sb[:])
                    nc.scalar.activation(krow_sb[:], pqt[0:1, :], AF.Copy)

                def a1_stage2():
                    nc.tensor.matmul(pbc, ones32_sb[:], krow_sb[:],
                                     start=True, stop=True)
                    nc.scalar.activation(tanh_sb[:], pbc, AF.Tanh,
                                         scale=-1.0, bias=qcol_sb[:])
                    nc.vector.tensor_scalar_mul(tanh_sb[:], tanh_sb[:],
                                                alpha_sb[:])
                    nc.vector.tensor_add(a1_sb[:], tanh_sb[:], adj_sb[:])

                # ---- woven PE stream: the previous batch's hs2/stats
                # flow through the aux bank inside this batch's DMA-paced
                # pool0 windows ----
                pool0_group(0, 0, SK)
                copy_hs0T(0)
                pool0_group(1, 0, 4)
                if b > 0:
                    emit_h2_aux(b - 1, 0)
                tr_chunk(0); tr_chunk(1)
                pool0_group(1, 4, 8)
                if b > 0:
                    emit_h2_aux(b - 1, 1)
                tr_chunk(2); tr_chunk(3)
                pool0_group(1, 8, 12)
                if b > 0:
                    emit_h2_aux(b - 1, 2)
                    conv1_chunk(0)
                pool0_group(1, 12, SK)
                copy_hs0T(1)
                if b > 0:
                    conv1_chunk(1)
                pool0_group(2, 0, 4)
                if b > 0:
                    emit_stats_aux(b - 1, 0)
                tr_chunk(4); tr_chunk(5)
                if b > 0:
                    conv1_chunk(2)
                pool0_group(2, 4, 8)
                if b > 0:
                    emit_stats_aux(b - 1, 1)
                tr_chunk(6); tr_chunk(7)
                if b > 0:
                    conv1_chunk(3)
                pool0_group(2, 8, 12)
                if b > 0:
                    emit_stats_aux(b - 1, 2)
                    conv1_chunk(4)
                pool0_group(2, 12, SK)
                copy_hs0T(2)
                if b > 0:
                    conv1_chunk(5)
                    stats_dma(b - 1)
                tr_chunk(8); tr_chunk(9)
                if b == PB - 1:
                    # last batch: transposes + kq burst the moment copy2
                    # lands, so the serial kq2->pqt->krow->pbc->tanh->a1
                    # chain starts ~2us earlier and finishes under the
                    # remaining conv1 banks.
                    tr_chunk(10); tr_chunk(11)
                    kq_burst()
                    a1_stage0()
                    conv1_chunk(6)
                    conv1_chunk(7)
                else:
                    if b > 0:
                        conv1_chunk(6)
                    tr_chunk(10); tr_chunk(11)
                    conv1_chunk(0 if b == 0 else 7)
                    kq_burst()
                    a1_stage0()
                if b < PB - 1:
                    conv1_chunk(1 if b == 0 else 8)
                    a1_stage1()
                    conv1_chunk(2 if b == 0 else 9)
                    a1_stage2()
                    for k in range(3 if b == 0 else 10, CK):
                        conv1_chunk(k)
                    if with_bc1:
                        for n in range(NK):
                            nc.tensor.matmul(pcs[n][:], ones16_sb[:],
                                             bc1_sb[:, n * 512:(n + 1) * 512],
                                             start=False, stop=True)
                    # hs1T PSUM->SBUF copies spread across engines
                    nc.scalar.activation(hs1T_sb[:, 0:512], pcs[0][:], AF.Copy)
                    nc.vector.tensor_copy(hs1T_sb[:, 512:1024], pcs[1][:])
                    nc.scalar.activation(hs1T_sb[:, 1024:1536], pcs[2][:],
                                         AF.Copy)
                    state[b] = (hs1T_sb, a1_sb)
                    return

                # ---- last batch: finish conv1 bank-major, pipelining BOTH
                # this batch's and the previous batch's hs2/stats tails into
                # the remaining banks so the engine queues drain in parallel
                # and almost no work is left after the final conv1 matmul ----
                state[b] = (hs1T_sb, a1_sb)

                def bank_finish(n):
                    for k in range(8, CK):
                        nc.tensor.matmul(
                            pcs[n][:], hs0_sb[:, k * NJ:(k + 1) * NJ],
                            wc1_sb[:, k * PS + n * 512: k * PS + n * 512 + 512],
                            start=False,
                            stop=(not with_bc1 and k == CK - 1))
                    if with_bc1:
                        nc.tensor.matmul(pcs[n][:], ones16_sb[:],
                                         bc1_sb[:, n * 512:(n + 1) * 512],
                                         start=False, stop=True)

                def copy_hs1T(n):
                    sl = slice(n * 512, (n + 1) * 512)
                    if n == 0:
                        nc.scalar.activation(hs1T_sb[:, sl], pcs[n][:], AF.Copy)
                    else:
                        nc.vector.tensor_copy(hs1T_sb[:, sl], pcs[n][:])

                a1_stage1()
                bank_finish(0)
                a1_stage2()
                copy_hs1T(0)
                bank_finish(1)
                copy_hs1T(1)
                bank_finish(2)
                emit_h2(b, [0])
                copy_hs1T(2)
                emit_h2(b, [1])
                emit_stats_n(b, 0)
                emit_h2(b, [2])
                emit_stats_n(b, 1)
                emit_stats_n(b, 2)
                stats_dma(b, last=True)

            for b in range(PB):
                run_iter(b)

    nc.compile()
    return nc


def _get_nc(with_bc1):
    key = ("nc", with_bc1)
    if key not in _CACHE:
        _CACHE[key] = _build_nc(with_bc1)
    return _CACHE[key]


def kernel(x, w_pool0, b_pool0, adj1, w_q, b_q, w_k, b_k, alpha,
           w_c1, b_c1, gamma, beta, w_pool1, b_pool1, w_cls, b_cls):
    global LAST_EXEC_NS
    x = np.asarray(x, np.float32)

    # ---- host-side input prep (sharding + weight folding) ----
    # (B, S, PS) transpose, then n-block-major swizzle: row p holds
    # [xT[k*128+p, n*512:(n+1)*512] for n outer, k inner] concatenated
    xt = x.reshape(B, PS, S).transpose(0, 2, 1).astype(np.float16)  # (B,S,PS)
    xh = np.ascontiguousarray(
        xt.reshape(B, SK, 128, NK, 512).transpose(0, 2, 3, 1, 4)).reshape(
        B, 128, NK * NBLK)
    pT = np.ascontiguousarray(np.asarray(w_pool0, np.float32).T).astype(np.float16)
    u_q = (np.asarray(w_q, np.float32).sum(0) / QK)
    u_k = (np.asarray(w_k, np.float32).sum(0) / QK)
    ukq = np.stack([u_k, u_q], 1).astype(np.float16)                # (PS, 2)
    wc1T = np.ascontiguousarray(np.asarray(w_c1, np.float32).T).astype(np.float16)
    onesw1 = np.stack([np.ones(NJ, np.float32),
                       np.asarray(w_pool1, np.float32)[0]], 1).astype(np.float16)
    c0 = float(np.asarray(b_q, np.float32).mean()
               - np.asarray(b_k, np.float32).mean())

    common = {
        "pT": np.ascontiguousarray(
            pT.reshape(SK, 128, NJ).transpose(1, 0, 2)).reshape(128, SK * NJ),
        "wc1T": np.ascontiguousarray(
            wc1T.reshape(CK, 128, PS).transpose(1, 0, 2)).reshape(128, CK * PS),
        "ukq": np.ascontiguousarray(
            ukq.reshape(CK, 128, 2).transpose(1, 0, 2)).reshape(128, CK * 2),
        "onesw1": onesw1,
        "adj": np.asarray(adj1, np.float32),
        "ident": np.eye(128, dtype=np.float16),
        "ident32": np.eye(128, dtype=np.float32),
        "ones1_16": np.ones((1, 128), np.float16),
        "ones1_32": np.ones((1, 128), np.float32),
        "bc1": np.asarray(b_c1, np.float32)[None, :].astype(np.float16),
        "bp0": np.asarray(b_pool0, np.float32)[:, None],
        "w1col": np.ascontiguousarray(
            np.asarray(w_pool1, np.float32)[0][:, None]),
        "c0": np.full((128, 1), c0, np.float32),
        "alphac": np.full((128, 1), np.asarray(alpha, np.float32)[0], np.float32),
    }
    in_maps = []
    for c in range(NCORES):
        m = dict(common)
        m["xh"] = np.ascontiguousarray(xh[c * PB:(c + 1) * PB])
        in_maps.append(m)

    nc = _get_nc(bool(np.any(np.asarray(b_c1))))
    res = run_bass_kernel_spmd(nc, in_maps, list(range(NCORES)), trace=TRACE,
                               tmpdir=TMPDIR)
    LAST_EXEC_NS = res.exec_time_ns

    # ---- host epilogue: BN stats all-reduce + affine + classifier ----
    rss = np.stack([res.results[c]["rss_out"] for c in range(NCORES)])
    ssum = rss[:, :, 0, :].sum((0, 1)).astype(np.float64)
    r_all = rss[:, :, 1, :].reshape(B, PS)
    ssq = rss[:, :, 2, :].sum((0, 1)).astype(np.float64)
    n = B * NJ
    mean = ssum / n
    var = ssq / n - mean * mean
    s = np.asarray(gamma, np.float64) / np.sqrt(var + BN_EPS)
    t = np.asarray(beta, np.float64) - s * mean
    w1sum = float(np.asarray(w_pool1, np.float64)[0].sum())
    pooled = s[None, :] * r_all.astype(np.float64) \
        + (t * w1sum + float(np.asarray(b_pool1)[0]))[None, :]
    out = pooled @ np.asarray(w_cls, np.float64).T + np.asarray(b_cls, np.float64)
    return out.astype(np.float32)


# revision 58
# speedup vs baseline: 1.0411x; 1.0411x over previous
"""GCNCombiner Trainium2 kernel — 8-core batch-parallel Bass/Tile implementation.

Math (reference):
  hs0 = x_flat @ w_pool0.T + b_pool0          (B, PS, NJ)
  q1  = mean_o(w_q @ hs0 + b_q) = u_q . hs0 + mean(b_q)   (B, NJ)
  k1  likewise
  A1  = adj1 + tanh(q1[:,None] - k1[None,:]) * alpha      (B, NJ, NJ)
  hs1 = w_c1 @ hs0 + b_c1                     (B, PS, NJ)
  hs2 = hs1 @ A1                              (B, PS, NJ)
  BN over (b, j) per channel; pool with w_pool1; classifier.

Because BN is a per-channel affine map s*h+t, the final output only needs
  r[b,c]    = sum_j hs2[b,c,j] * w_pool1[j]
  ssum[c]   = sum_{b,j} hs2[b,c,j]
  ssq[c]    = sum_{b,j} hs2[b,c,j]^2
Each core computes these for its 4 batches; the 8-way reduction of
ssum/ssq (the BN batch-stats all-reduce) and the tiny (32x1536)@(1536x200)
classifier run on the host during the gather/unshard step.

Device schedule (v2): x is host-swizzled n-block-major so pool0 runs
n-outer/k-inner — each 512-wide output bank completes after 16 matmuls,
and its transposes + conv1 chunks pipeline INSIDE the rest of pool0
instead of serializing after it (batch 0 keeps conv1 at the tail since
wc1T is still in flight).  q/k ride conv1 as a 4th 2-column moving
chunk against the same stationary hs0 chunk.  The previous batch's
hs2/stats matmuls weave into this batch's stream as PE filler.  All
PSUM->SBUF copies alternate scalar/vector/gpsimd so no single engine's
queue gates the PE.  DMA order: pT, x0 (6 chunks), wc1T (12 chunks),
x1, x2, x3 — consumption order matches arrival order everywhere.
"""

import numpy as np

import concourse.bacc as bacc
import concourse.mybir as mybir
import concourse.tile as tile
from concourse.bass_utils import run_bass_kernel_spmd

# problem shapes (hardcoded per contract)
B, PS, H, W = 32, 1536, 32, 64
S = H * W                # 2048 selects
NJ = 128                 # joints
QK = PS // 4
NC = 200
BN_EPS = 1e-5

NCORES = 8
PB = B // NCORES         # batches per core = 4
SK = S // 128            # 16 s-chunks
CK = PS // 128           # 12 c-chunks
NK = PS // 512           # 3 free-dim chunks of 512
NBLK = SK * 512          # 8192 cols per n-block of x

F16 = mybir.dt.float16
F32 = mybir.dt.float32
AF = mybir.ActivationFunctionType

TRACE = False            # set True (e.g. from test.py) to profile via NTFF
LAST_EXEC_NS = None
TMPDIR = None
_CACHE = {}


def _build_nc(with_bc1=True):
    nc = bacc.Bacc("TRN2", target_bir_lowering=False, debug=False,
                   num_devices=NCORES)

    d = {}
    # layouts pre-swizzled on host so each SBUF partition's bytes are one
    # contiguous DRAM run (large DMA descriptors -> near-peak HBM bandwidth).
    # x is n-block-major: [b, p, n, k, 512] so pool0 can run n-outer.
    d["xh"] = nc.dram_tensor("xh", [PB, 128, NK * NBLK], F16,
                             kind="ExternalInput").ap()
    d["pT"] = nc.dram_tensor("pT", [128, SK * NJ], F16, kind="ExternalInput").ap()
    d["wc1T"] = nc.dram_tensor("wc1T", [128, CK * PS], F16,
                               kind="ExternalInput").ap()
    d["ukq"] = nc.dram_tensor("ukq", [128, CK * 2], F16, kind="ExternalInput").ap()
    d["onesw1"] = nc.dram_tensor("onesw1", [128, 2], F16, kind="ExternalInput").ap()
    d["adj"] = nc.dram_tensor("adj", [NJ, NJ], F32, kind="ExternalInput").ap()
    d["ident"] = nc.dram_tensor("ident", [128, 128], F16, kind="ExternalInput").ap()
    d["ident32"] = nc.dram_tensor("ident32", [128, 128], F32, kind="ExternalInput").ap()
    d["ones1_16"] = nc.dram_tensor("ones1_16", [1, 128], F16, kind="ExternalInput").ap()
    d["ones1_32"] = nc.dram_tensor("ones1_32", [1, 128], F32, kind="ExternalInput").ap()
    d["bc1"] = nc.dram_tensor("bc1", [1, PS], F16, kind="ExternalInput").ap()
    d["bp0"] = nc.dram_tensor("bp0", [128, 1], F32, kind="ExternalInput").ap()
    d["w1col"] = nc.dram_tensor("w1col", [128, 1], F32, kind="ExternalInput").ap()
    d["c0"] = nc.dram_tensor("c0", [128, 1], F32, kind="ExternalInput").ap()
    d["alphac"] = nc.dram_tensor("alphac", [128, 1], F32, kind="ExternalInput").ap()

    # per batch: [r, ssum, ssq] concatenated along the free dim
    rss_out = nc.dram_tensor("rss_out", [PB, 3, PS], F32,
                             kind="ExternalOutput").ap()

    with tile.TileContext(nc) as tc:
        with tc.tile_pool(name="const", bufs=1) as cp, \
             tc.tile_pool(name="xp", bufs=2) as xp, \
             tc.tile_pool(name="work", bufs=2) as wp, \
             tc.tile_pool(name="sm", bufs=2) as smp, \
             tc.tile_pool(name="rp", bufs=2) as rp, \
             tc.tile_pool(name="h2p", bufs=3) as wph, \
             tc.tile_pool(name="mm", bufs=3, space="PSUM") as pmm, \
             tc.tile_pool(name="cv", bufs=3, space="PSUM") as pcv, \
             tc.tile_pool(name="tr", bufs=1, space="PSUM") as ptr, \
             tc.tile_pool(name="aux", bufs=1, space="PSUM") as paux:

            # ---- DMA order matters: pT first, then batch-0 x, then wc1T
            # (12 chunk-DMAs for fine-grained conv1 pacing), then x1..x3 ----
            pT_sb = cp.tile([128, SK * NJ], F16, tag="pT")
            nc.sync.dma_start(out=pT_sb[:], in_=d["pT"])

            # batch-0 x and wc1 alternate between the sync and scalar HWDGE
            # rings: two descriptor-generator streams fill the 16 DMA engines
            # faster during the cold-start ramp.
            x_tiles = []
            x0 = xp.tile([128, NK * NBLK], F16, tag="x", name="x_sb_pre0")
            hf = NBLK // 4
            for ci in range(12):
                nc.sync.dma_start(out=x0[:, ci * hf:(ci + 1) * hf],
                                  in_=d["xh"][0, :, ci * hf:(ci + 1) * hf])
            x_tiles.append(x0)

            wc1_sb = cp.tile([128, CK * PS], F16, tag="wc1")
            for k in range(CK):
                nc.sync.dma_start(out=wc1_sb[:, k * PS:(k + 1) * PS],
                                  in_=d["wc1T"][:, k * PS:(k + 1) * PS])

            for b in range(1, PB):
                xb = xp.tile([128, NK * NBLK], F16, tag="x", name=f"x_sb{b}")
                for ci in range(12):
                    nc.sync.dma_start(out=xb[:, ci * hf:(ci + 1) * hf],
                                      in_=d["xh"][b, :, ci * hf:(ci + 1) * hf])
                x_tiles.append(xb)

            # small constants ride the gpsimd (SWDGE) queue in parallel
            ukq_sb = cp.tile([128, CK * 2], F16, tag="ukq")
            nc.gpsimd.dma_start(out=ukq_sb[:], in_=d["ukq"])
            onesw1_sb = cp.tile([128, 2], F16, tag="onesw1")
            nc.gpsimd.dma_start(out=onesw1_sb[:], in_=d["onesw1"])
            adj_sb = cp.tile([NJ, NJ], F32, tag="adj")
            nc.gpsimd.dma_start(out=adj_sb[:], in_=d["adj"])
            ident_sb = cp.tile([128, 128], F16, tag="ident")
            nc.gpsimd.dma_start(out=ident_sb[:], in_=d["ident"])
            ident32_sb = cp.tile([128, 128], F32, tag="ident32")
            nc.gpsimd.dma_start(out=ident32_sb[:], in_=d["ident32"])
            ones16_sb = cp.tile([1, 128], F16, tag="ones16")
            nc.gpsimd.dma_start(out=ones16_sb[:], in_=d["ones1_16"])
            ones32_sb = cp.tile([1, 128], F32, tag="ones32")
            nc.gpsimd.dma_start(out=ones32_sb[:], in_=d["ones1_32"])
            bc1_sb = cp.tile([1, PS], F16, tag="bc1")
            nc.gpsimd.dma_start(out=bc1_sb[:], in_=d["bc1"])
            bp0_sb = cp.tile([128, 1], F32, tag="bp0")
            nc.gpsimd.dma_start(out=bp0_sb[:], in_=d["bp0"])
            w1col_sb = cp.tile([128, 1], F32, tag="w1col")
            nc.gpsimd.dma_start(out=w1col_sb[:], in_=d["w1col"])
            c0_sb = cp.tile([128, 1], F32, tag="c0")
            nc.gpsimd.dma_start(out=c0_sb[:], in_=d["c0"])
            alpha_sb = cp.tile([128, 1], F32, tag="alphac")
            nc.gpsimd.dma_start(out=alpha_sb[:], in_=d["alphac"])

            # HAM warmup: the PE would otherwise idle ~9us waiting for the
            # first DMAs; dummy matmuls on a memset tile bring the clock
            # gate up before the real stream starts.
            wu_sb = cp.tile([128, 512], F16, tag="wu")
            nc.vector.memset(wu_sb[:], 0.0)
            for wi in range(24):
                pw = pmm.tile([128, 512], F32, tag="mmt", name=f"wu{wi}")
                nc.tensor.matmul(pw[:], wu_sb[:, 0:128], wu_sb[:],
                                 start=True, stop=True)

            # single-bank PSUM scratch: transposes rotate through 8 slices
            # of one persistent [128,1024]f16 tile; the small matmul outputs
            # (pkq accum, k-row broadcast, kq transpose) carve disjoint
            # column ranges of one persistent [128,512]f32 tile.  Sub-tile
            # range tracking gives each slice independent dependencies.
            trbig = ptr.tile([128, 1024], F16, tag="trt", name="trbig")
            auxt = paux.tile([128, 512], F32, tag="auxt", name="auxt")
            tr_slot = [0]

            state = [None] * PB     # (hs1T_sb, a1_sb) per batch
            h2state = [None] * PB   # (h2cs, sqcs) per batch

            def emit_h2(b, nlist=range(NK)):
                """hs2T chunk matmuls + PSUM->SBUF copies for batch b.  Each
                chunk's h2/square copies ride different engines so a chunk is
                consumable ~0.7us after its matmul."""
                hs1T_sb, a1_sb = state[b]
                if h2state[b] is None:
                    h2state[b] = ([None] * NK, [None] * NK)
                h2cs, sqcs = h2state[b]
                for n in nlist:
                    ph = pmm.tile([128, 512], F32, tag="mmt", name=f"h2_{b}_{n}")
                    nc.tensor.matmul(ph[:], a1_sb[:],
                                     hs1T_sb[:, n * 512:(n + 1) * 512],
                                     start=True, stop=True)
                    h2_sb = wph.tile([128, 512], F16, tag="h2c",
                                     name=f"h2c{b}_{n}")
                    sq_sb = wph.tile([128, 512], F16, tag="sqc",
                                     name=f"sqc{b}_{n}")
                    if n % 2 == 0:
                        nc.vector.tensor_copy(h2_sb[:], ph[:])
                        nc.scalar.activation(sq_sb[:], ph[:], AF.Square)
                    else:
                        nc.scalar.activation(h2_sb[:], ph[:], AF.Copy)
                        nc.vector.tensor_tensor(sq_sb[:], h2_sb[:], h2_sb[:],
                                                mybir.AluOpType.mult)
                    h2cs[n] = h2_sb
                    sqcs[n] = sq_sb

            _stats_sb = {}

            def emit_stats_n(b, n):
                h2cs, sqcs = h2state[b]
                if b not in _stats_sb:
                    _stats_sb[b] = (
                        rp.tile([2, PS], F32, tag="sr", name=f"sr{b}"),
                        rp.tile([1, PS], F32, tag="ssq", name=f"ssq{b}"))
                sr_sb, ssq_sb = _stats_sb[b]
                sl = slice(n * 512, (n + 1) * 512)
                prs = pmm.tile([2, 512], F32, tag="mmt", name=f"prs{b}_{n}")
                nc.tensor.matmul(prs[:], onesw1_sb[:], h2cs[n][:],
                                 start=True, stop=True)
                pq2 = pmm.tile([1, 512], F32, tag="mmt", name=f"pq2{b}_{n}")
                nc.tensor.matmul(pq2[:], onesw1_sb[:, 0:1], sqcs[n][:],
                                 start=True, stop=True)
                nc.vector.tensor_copy(sr_sb[:, sl], prs[:])
                nc.scalar.activation(ssq_sb[:, sl], pq2[:], AF.Copy)

            def stats_dma_slice(b, n):
                """Per-slice output DMA for the last batch: descriptors are
                generated while the PE still runs, and the final pair rides
                two queues in parallel."""
                sr_sb, ssq_sb = _stats_sb[b]
                sl = slice(n * 512, (n + 1) * 512)
                nc.sync.dma_start(out=rss_out[b, 0:2, sl], in_=sr_sb[:, sl])
                nc.gpsimd.dma_start(out=rss_out[b, 2:3, sl], in_=ssq_sb[:, sl])

            def stats_dma(b, last=False):
                sr_sb, ssq_sb = _stats_sb[b]
                # rss_out rows: 0 = ssum, 1 = r, 2 = ssq (sr rows are [ssum, r])
                eng = nc.sync if last else nc.gpsimd
                eng.dma_start(out=rss_out[b, 0:2, :], in_=sr_sb[:])
                eng.dma_start(out=rss_out[b, 2:3, :], in_=ssq_sb[:])

            def emit_stats(b):
                """r/ssum/ssq reductions + output DMA for batch b."""
                for n in range(NK):
                    emit_stats_n(b, n)
                stats_dma(b)

            def emit_h2_aux(b, n):
                """Single hs2T chunk routed through the shared aux PSUM bank:
                lets the previous batch's tail run inside this batch's
                DMA-paced pool0 windows without extra PSUM banks."""
                hs1T_sb, a1_sb = state[b]
                if h2state[b] is None:
                    h2state[b] = ([None] * NK, [None] * NK)
                h2cs, sqcs = h2state[b]
                ph = auxt[:]
                nc.tensor.matmul(ph, a1_sb[:],
                                 hs1T_sb[:, n * 512:(n + 1) * 512],
                                 start=True, stop=True)
                h2_sb = wph.tile([128, 512], F16, tag="h2c", name=f"h2c{b}_{n}")
                sq_sb = wph.tile([128, 512], F16, tag="sqc", name=f"sqc{b}_{n}")
                if n % 2 == 0:
                    nc.vector.tensor_copy(h2_sb[:], ph)
                    nc.scalar.activation(sq_sb[:], ph, AF.Square)
                else:
                    nc.scalar.activation(h2_sb[:], ph, AF.Copy)
                    nc.vector.tensor_tensor(sq_sb[:], h2_sb[:], h2_sb[:],
                                            mybir.AluOpType.mult)
                h2cs[n] = h2_sb
                sqcs[n] = sq_sb

            def emit_stats_aux(b, n):
                h2cs, sqcs = h2state[b]
                if b not in _stats_sb:
                    _stats_sb[b] = (
                        rp.tile([2, PS], F32, tag="sr", name=f"sr{b}"),
                        rp.tile([1, PS], F32, tag="ssq", name=f"ssq{b}"))
                sr_sb, ssq_sb = _stats_sb[b]
                sl = slice(n * 512, (n + 1) * 512)
                prs = auxt[0:2, :]
                nc.tensor.matmul(prs, onesw1_sb[:], h2cs[n][:],
                                 start=True, stop=True)
                pq2 = auxt[32:33, :]
                nc.tensor.matmul(pq2, onesw1_sb[:, 0:1], sqcs[n][:],
                                 start=True, stop=True)
                nc.vector.tensor_copy(sr_sb[:, sl], prs)
                nc.scalar.activation(ssq_sb[:, sl], pq2, AF.Copy)



            def run_iter(b):
                """One batch: n-outer pool0 with transposes (and for b>0 the
                conv1 chunks + previous batch's tail) woven into the stream."""
                x_sb = x_tiles[b]

                hs0T_cs = [None] * NK     # SBUF hs0T 512-col groups
                hs0_sb = wp.tile([128, CK * NJ], F16, tag="hs0", name=f"hs0_{b}")
                hs1T_sb = wp.tile([128, PS], F16, tag="hs1T", name=f"hs1T{b}")
                pcs = [pcv.tile([128, 512], F32, tag="cvt", name=f"c1_{b}_{n}")
                       for n in range(NK)]
                pkq = auxt[:, 0:2]
                pss = [pmm.tile([128, 512], F32, tag="mmt", name=f"p0_{b}_{n}")
                       for n in range(NK)]

                def pool0_group(n, klo, khi):
                    for k in range(klo, khi):
                        nc.tensor.matmul(
                            pss[n][:],
                            pT_sb[:, k * NJ:(k + 1) * NJ],
                            x_sb[:, n * NBLK + k * 512: n * NBLK + k * 512 + 512],
                            start=(k == 0), stop=(k == SK - 1))

                def copy_hs0T(n):
                    t = wp.tile([128, 512], F16, tag=f"hs0T{n}",
                                name=f"hs0T{b}_{n}")
                    # fold b_pool0 (per-partition j) into the PSUM->SBUF copy
                    nc.vector.tensor_scalar_add(t[:], pss[n][:], bp0_sb[:])
                    hs0T_cs[n] = t

                def tr_chunk(k):
                    s = tr_slot[0] % 8
                    tr_slot[0] += 1
                    pt = trbig[:, s * 128:(s + 1) * 128]
                    nc.tensor.transpose(
                        pt,
                        hs0T_cs[k // 4][:, (k % 4) * 128:(k % 4) * 128 + 128],
                        ident_sb[:])
                    if k % 2 == 0:
                        nc.vector.tensor_copy(hs0_sb[:, k * NJ:(k + 1) * NJ], pt)
                    else:
                        nc.scalar.activation(hs0_sb[:, k * NJ:(k + 1) * NJ],
                                             pt, AF.Copy)

                def conv1_chunk(k):
                    st = hs0_sb[:, k * NJ:(k + 1) * NJ]
                    for n in range(NK):
                        nc.tensor.matmul(
                            pcs[n][:], st,
                            wc1_sb[:, k * PS + n * 512: k * PS + n * 512 + 512],
                            start=(k == 0),
                            stop=(not with_bc1 and k == CK - 1))

                def kq_burst():
                    # q/k projections: 12 cheap 2-col matmuls on the same
                    # stationaries conv1 uses; run as one burst the moment
                    # the last transpose lands so the A1 chain can overlap
                    # the remaining conv1 chunks.
                    for k in range(CK):
                        nc.tensor.matmul(pkq, hs0_sb[:, k * NJ:(k + 1) * NJ],
                                         ukq_sb[:, 2 * k:2 * k + 2],
                                         start=(k == 0), stop=(k == CK - 1))

                # ---- A1 chain pieces (PE ops woven into conv1 below) ----
                kq2_sb = smp.tile([128, 2], F32, tag="kq2", name=f"kq2{b}")
                qcol_sb = smp.tile([128, 1], F32, tag="qcol", name=f"qcol{b}")
                krow_sb = smp.tile([1, 128], F32, tag="krow", name=f"krow{b}")
                tanh_sb = smp.tile([128, 128], F32, tag="tanh", name=f"tanh{b}")
                a1_sb = smp.tile([NJ, NJ], F16, tag="a1", name=f"a1_{b}")
                pqt = auxt[0:2, 130:258]
                pbc = auxt[:, 2:130]

                def a1_stage0():
                    # pkq cols: [k0, q0]; qcol = q0 + (mean(b_q)-mean(b_k))
                    nc.vector.tensor_copy(kq2_sb[:], pkq)
                    nc.scalar.activation(qcol_sb[:], pkq[:, 1:2], AF.Identity,
                                         bias=c0_sb[:])

                def a1_stage1():
                    nc.tensor.transpose(pqt, kq2_sb[:], ident32_sb[:])
                    nc.scalar.activation(krow_sb[:], pqt[0:1, :], AF.Copy)

                def a1_stage2():
                    nc.tensor.matmul(pbc, ones32_sb[:], krow_sb[:],
                                     start=True, stop=True)
                    nc.scalar.activation(tanh_sb[:], pbc, AF.Tanh,
                                         scale=-1.0, bias=qcol_sb[:])
                    nc.vector.tensor_scalar_mul(tanh_sb[:], tanh_sb[:],
                                                alpha_sb[:])
                    nc.vector.tensor_add(a1_sb[:], tanh_sb[:], adj_sb[:])

                # ---- woven PE stream: the previous batch's hs2/stats
                # flow through the aux bank inside this batch's DMA-paced
                # pool0 windows ----
                pool0_group(0, 0, SK)
                copy_hs0T(0)
                pool0_group(1, 0, 4)
                if b > 0:
                    emit_h2_aux(b - 1, 0)
                tr_chunk(0); tr_chunk(1)
                pool0_group(1, 4, 8)
                if b > 0:
                    emit_h2_aux(b - 1, 1)
                tr_chunk(2); tr_chunk(3)
                pool0_group(1, 8, 12)
                if b > 0:
                    emit_h2_aux(b - 1, 2)
                    conv1_chunk(0)
                pool0_group(1, 12, SK)
                copy_hs0T(1)
                if b > 0:
                    conv1_chunk(1)
                pool0_group(2, 0, 4)
                if b > 0:
                    emit_stats_aux(b - 1, 0)
                tr_chunk(4); tr_chunk(5)
                if b > 0:
                    conv1_chunk(2)
                pool0_group(2, 4, 8)
                if b > 0:
                    emit_stats_aux(b - 1, 1)
                tr_chunk(6); tr_chunk(7)
                if b > 0:
                    conv1_chunk(3)
                pool0_group(2, 8, 12)
                if b > 0:
                    emit_stats_aux(b - 1, 2)
                    conv1_chunk(4)
                pool0_group(2, 12, SK)
                copy_hs0T(2)
                if b > 0:
                    conv1_chunk(5)
                    stats_dma(b - 1)
                tr_chunk(8); tr_chunk(9)
                if b > 0:
                    conv1_chunk(6)
                tr_chunk(10); tr_chunk(11)
                conv1_chunk(0 if b == 0 else 7)
                kq_burst()
                a1_stage0()
                if b < PB - 1:
                    conv1_chunk(1 if b == 0 else 8)
                    a1_stage1()
                    conv1_chunk(2 if b == 0 else 9)
                    a1_stage2()
                    for k in range(3 if b == 0 else 10, CK):
                        conv1_chunk(k)
                    if with_bc1:
                        for n in range(NK):
                            nc.tensor.matmul(pcs[n][:], ones16_sb[:],
                                             bc1_sb[:, n * 512:(n + 1) * 512],
                                             start=False, stop=True)
                    # hs1T PSUM->SBUF copies spread across engines
                    nc.scalar.activation(hs1T_sb[:, 0:512], pcs[0][:], AF.Copy)
                    nc.vector.tensor_copy(hs1T_sb[:, 512:1024], pcs[1][:])
                    nc.scalar.activation(hs1T_sb[:, 1024:1536], pcs[2][:],
                                         AF.Copy)
                    state[b] = (hs1T_sb, a1_sb)
                    return

                # ---- last batch: finish conv1 bank-major, pipelining BOTH
                # this batch's and the previous batch's hs2/stats tails into
                # the remaining banks so the engine queues drain in parallel
                # and almost no work is left after the final conv1 matmul ----
                state[b] = (hs1T_sb, a1_sb)

                def bank_finish(n):
                    for k in range(8, CK):
                        nc.tensor.matmul(
                            pcs[n][:], hs0_sb[:, k * NJ:(k + 1) * NJ],
                            wc1_sb[:, k * PS + n * 512: k * PS + n * 512 + 512],
                            start=False,
                            stop=(not with_bc1 and k == CK - 1))
                    if with_bc1:
                        nc.tensor.matmul(pcs[n][:], ones16_sb[:],
                                         bc1_sb[:, n * 512:(n + 1) * 512],
                                         start=False, stop=True)

                def copy_hs1T(n):
                    sl = slice(n * 512, (n + 1) * 512)
                    if n == 0:
                        nc.scalar.activation(hs1T_sb[:, sl], pcs[n][:], AF.Copy)
                    else:
                        nc.vector.tensor_copy(hs1T_sb[:, sl], pcs[n][:])

                bank_finish(0)
                a1_stage1()
                bank_finish(1)
                a1_stage2()
                copy_hs1T(0)
                copy_hs1T(1)
                bank_finish(2)
                emit_h2(b, [0])
                copy_hs1T(2)
                emit_h2(b, [1])
                emit_stats_n(b, 0)
                emit_h2(b, [2])
                emit_stats_n(b, 1)
                emit_stats_n(b, 2)
                stats_dma(b, last=True)

            for b in range(PB):
                run_iter(b)

    nc.compile()
    return nc


def _get_nc(with_bc1):
    key = ("nc", with_bc1)
    if key not in _CACHE:
        _CACHE[key] = _build_nc(with_bc1)
    return _CACHE[key]


def kernel(x, w_pool0, b_pool0, adj1, w_q, b_q, w_k, b_k, alpha,
           w_c1, b_c1, gamma, beta, w_pool1, b_pool1, w_cls, b_cls):
    global LAST_EXEC_NS
    x = np.asarray(x, np.float32)

    # ---- host-side input prep (sharding + weight folding) ----
    # (B, S, PS) transpose, then n-block-major swizzle: row p holds
    # [xT[k*128+p, n*512:(n+1)*512] for n outer, k inner] concatenated
    xt = x.reshape(B, PS, S).transpose(0, 2, 1).astype(np.float16)  # (B,S,PS)
    xh = np.ascontiguousarray(
        xt.reshape(B, SK, 128, NK, 512).transpose(0, 2, 3, 1, 4)).reshape(
        B, 128, NK * NBLK)
    pT = np.ascontiguousarray(np.asarray(w_pool0, np.float32).T).astype(np.float16)
    u_q = (np.asarray(w_q, np.float32).sum(0) / QK)
    u_k = (np.asarray(w_k, np.float32).sum(0) / QK)
    ukq = np.stack([u_k, u_q], 1).astype(np.float16)                # (PS, 2)
    wc1T = np.ascontiguousarray(np.asarray(w_c1, np.float32).T).astype(np.float16)
    onesw1 = np.stack([np.ones(NJ, np.float32),
                       np.asarray(w_pool1, np.float32)[0]], 1).astype(np.float16)
    c0 = float(np.asarray(b_q, np.float32).mean()
               - np.asarray(b_k, np.float32).mean())

    common = {
        "pT": np.ascontiguousarray(
            pT.reshape(SK, 128, NJ).transpose(1, 0, 2)).reshape(128, SK * NJ),
        "wc1T": np.ascontiguousarray(
            wc1T.reshape(CK, 128, PS).transpose(1, 0, 2)).reshape(128, CK * PS),
        "ukq": np.ascontiguousarray(
            ukq.reshape(CK, 128, 2).transpose(1, 0, 2)).reshape(128, CK * 2),
        "onesw1": onesw1,
        "adj": np.asarray(adj1, np.float32),
        "ident": np.eye(128, dtype=np.float16),
        "ident32": np.eye(128, dtype=np.float32),
        "ones1_16": np.ones((1, 128), np.float16),
        "ones1_32": np.ones((1, 128), np.float32),
        "bc1": np.asarray(b_c1, np.float32)[None, :].astype(np.float16),
        "bp0": np.asarray(b_pool0, np.float32)[:, None],
        "w1col": np.ascontiguousarray(
            np.asarray(w_pool1, np.float32)[0][:, None]),
        "c0": np.full((128, 1), c0, np.float32),
        "alphac": np.full((128, 1), np.asarray(alpha, np.float32)[0], np.float32),
    }
    in_maps = []
    for c in range(NCORES):
        m = dict(common)
        m["xh"] = np.ascontiguousarray(xh[c * PB:(c + 1) * PB])
        in_maps.append(m)

    nc = _get_nc(bool(np.any(np.asarray(b_c1))))
    res = run_bass_kernel_spmd(nc, in_maps, list(range(NCORES)), trace=TRACE,
                               tmpdir=TMPDIR)
    LAST_EXEC_NS = res.exec_time_ns

    # ---- host epilogue: BN stats all-reduce + affine + classifier ----
    rss = np.stack([res.results[c]["rss_out"] for c in range(NCORES)])
    ssum = rss[:, :, 0, :].sum((0, 1)).astype(np.float64)
    r_all = rss[:, :, 1, :].reshape(B, PS)
    ssq = rss[:, :, 2, :].sum((0, 1)).astype(np.float64)
    n = B * NJ
    mean = ssum / n
    var = ssq / n - mean * mean
    s = np.asarray(gamma, np.float64) / np.sqrt(var + BN_EPS)
    t = np.asarray(beta, np.float64) - s * mean
    w1sum = float(np.asarray(w_pool1, np.float64)[0].sum())
    pooled = s[None, :] * r_all.astype(np.float64) \
        + (t * w1sum + float(np.asarray(b_pool1)[0]))[None, :]
    out = pooled @ np.asarray(w_cls, np.float64).T + np.asarray(b_cls, np.float64)
    return out.astype(np.float32)
